# revision 15
# baseline (speedup 1.0000x reference)
"""EuclideanDeconf kernel for 8x TRN2 NeuronCores.

Computes out[b, c] = (2/D) * x @ W.T - ||x||^2/D - ||W||^2/D
for x [16384, 1024] f32, W [2048, 1024] f32 -> out [16384, 2048] f32.

Sharding: data-parallel over the batch dim. Each of the 8 cores gets 2048
rows of x (passed pre-transposed as xT [1024, 2048] f32) and the full W
(passed pre-transposed and bf16-cast as wT [1024, 2048] bf16). The host does
layout-only work (transpose / cast / shard / concat); all FLOPs (matmul,
row/col norms, combine) run on device.

Numerics: cross term in bf16 (its magnitude is ~0.003 of the ~1.0 output, so
bf16 rounding contributes ~1e-5 relative error); x2 computed on-device in
fp32 from the fp32 x (the dominant term, kept exact); w2 from bf16 W (w2 is
~0.002 so its rounding is ~1e-5 absolute).

Engine assignment (per core, steady state):
  PE:     512 bf16 matmuls [128x128]x[128x512] (the 8.6 GFLOP core of the op)
          + 32 w2-reduce matmuls + 16 tiny x2-dot matmuls
  ACT:    epilogue pass 1: t = (2/D)*psum - x2[b]  (scale + partition bias)
  DVE:    W^2 squares, x2 k-add-tree, epilogue pass 2: y = t - w2[c]
  GPSIMD: x f32->bf16 cast, x^2 squares, w2 partition_broadcast
  DMA:    x + y on the SP (sync) HWDGE ring, W on the ACT (scalar) ring
"""

import numpy as np
import ml_dtypes

# Problem constants (hardcoded; kernel.py must be self-contained).
B, D, C = 16384, 1024, 2048
NCORES = 8
BSH = B // NCORES  # 2048 rows of x per core
P = 128            # partitions
KT = D // P        # 8 contraction tiles
BCH = 512          # b-chunk (columns of xT loaded per DMA)
NTB = BSH // P     # 16 b-tiles per core

_CACHE = {}

# bisection knobs (import os at module level keeps kernel self-contained)
import os as _os

_W_RING = _os.environ.get("K_W_RING", "scalar")    # scalar | sync
_WSQ_ENG = _os.environ.get("K_WSQ", "vector")      # vector | scalar
_XSQ_ENG = _os.environ.get("K_XSQ", "gpsimd")      # gpsimd | vector
_CAST_ENG = _os.environ.get("K_CAST", "gpsimd")    # gpsimd | vector
_BCAST = _os.environ.get("K_BCAST", "gpsimd")      # gpsimd | pe


def _WDMA_ENGINE(nc):
    return nc.scalar if _W_RING == "scalar" else nc.sync


def _build_nc():
    import concourse.tile as tile
    import concourse.mybir as mybir
    import concourse.bass as bass
    from concourse import bacc

    f32 = mybir.dt.float32
    bf16 = mybir.dt.bfloat16
    PSUM = bass.MemorySpace.PSUM
    Identity = mybir.ActivationFunctionType.Identity
    MULT = mybir.AluOpType.mult

    nc = bacc.Bacc(
        "TRN2",
        target_bir_lowering=False,
        debug=False,
        enable_asserts=False,
    )
    xT = nc.dram_tensor("xT", [D, BSH], f32, kind="ExternalInput").ap()
    wT = nc.dram_tensor("wT", [D, C], bf16, kind="ExternalInput").ap()
    y = nc.dram_tensor("y", [BSH, C], f32, kind="ExternalOutput").ap()

    with tile.TileContext(nc) as tc:
        with (
            tc.tile_pool(name="consts", bufs=1) as cpool,
            tc.tile_pool(name="wpool", bufs=1) as wpool,
            tc.tile_pool(name="xpool", bufs=2) as xpool,
            tc.tile_pool(name="xsqpool", bufs=2) as xsqpool,
            tc.tile_pool(name="epool", bufs=3) as epool,
            tc.tile_pool(name="ypool", bufs=2) as ypool,
            tc.tile_pool(name="spool", bufs=2) as spool,
            tc.tile_pool(name="pmain", bufs=3, space=PSUM) as pmain,
            tc.tile_pool(name="psmall", bufs=1, space=PSUM) as psmall,
        ):
            negones_f = cpool.tile([P, 1], f32)
            nc.gpsimd.memset(negones_f[:], -1.0)
            negones_b = cpool.tile([P, 1], bf16)
            nc.gpsimd.memset(negones_b[:], -1.0)

            # --- Load W (bf16, transposed), split per k-tile on the ACT ring ---
            wbf = wpool.tile([P, KT, C], bf16)
            wTr = wT.rearrange("(k p) c -> p k c", p=P)
            for k in range(KT):
                _WDMA_ENGINE(nc).dma_start(wbf[:, k, :], wTr[:, k, :])

            # --- w2[c] = ||W[c]||^2 / D, negated, replicated to [128, C] ---
            # squares on DVE (early, engine otherwise idle), partition-sum on
            # PE via -1s stationary, broadcast on GPSIMD.
            w2rep = wpool.tile([P, C], f32)
            wsq = wpool.tile([P, KT, C], bf16)
            _wsq_eng = nc.vector if _WSQ_ENG == "vector" else nc.scalar
            for k in range(KT):
                if _WSQ_ENG == "scalar":
                    nc.scalar.activation(
                        wsq[:, k, :], wbf[:, k, :],
                        mybir.ActivationFunctionType.Square,
                    )
                else:
                    nc.vector.tensor_tensor(
                        wsq[:, k, :], wbf[:, k, :], wbf[:, k, :], op=MULT
                    )
            w2row = wpool.tile([1, C], f32)
            for cj in range(C // 512):
                w2ps = psmall.tile([1, 512], f32, tag="w2ps", bufs=1)
                for k in range(KT):
                    nc.tensor.matmul(
                        w2ps[:],
                        negones_b[:],
                        wsq[:, k, cj * 512:(cj + 1) * 512],
                        start=(k == 0),
                        stop=(k == KT - 1),
                    )
                # psum holds -sum W^2; scale by 1/D during the copy out
                nc.vector.tensor_scalar_mul(
                    w2row[:, cj * 512:(cj + 1) * 512], w2ps[:], 1.0 / D
                )
            if _BCAST == "gpsimd":
                nc.gpsimd.partition_broadcast(w2rep[:], w2row[:], channels=P)
            else:
                ones1_f = cpool.tile([1, P], f32)
                nc.gpsimd.memset(ones1_f[:], 1.0)
                for cj in range(C // 512):
                    w2rp = psmall.tile([P, 512], f32, tag="w2ps", bufs=1)
                    nc.tensor.matmul(
                        w2rp[:], ones1_f[:], w2row[:, cj * 512:(cj + 1) * 512],
                        start=True, stop=True,
                    )
                    nc.vector.tensor_copy(w2rep[:, cj * 512:(cj + 1) * 512], w2rp[:])

            # --- Main loop over batch chunks of 512 rows ---
            for ch in range(BSH // BCH):
                xf = xpool.tile([P, KT, BCH], f32, tag="xf")
                xbf = xpool.tile([P, KT, BCH], bf16, tag="xbf")
                xTr = xT[:, ch * BCH:(ch + 1) * BCH].rearrange(
                    "(k p) b -> p k b", p=P
                )
                _cast_eng = nc.gpsimd if _CAST_ENG == "gpsimd" else nc.vector
                if ch == 0:
                    # fine-grained first load so PE can start ASAP
                    for k in range(KT):
                        nc.sync.dma_start(xf[:, k, :], xTr[:, k, :])
                        _cast_eng.tensor_copy(xbf[:, k, :], xf[:, k, :])
                else:
                    nc.sync.dma_start(xf[:], xTr)
                    _cast_eng.tensor_copy(xbf[:], xf[:])

                # x2 column (negated) for the 4 b-tiles of this chunk:
                # square (gpsimd), k-add-tree (DVE), partition-dot (PE).
                x2ps = psmall.tile([P, 4], f32, tag="x2ps", bufs=1)
                x2cols = spool.tile([P, 4], f32, tag="x2cols")
                for jj in range(BCH // P):
                    sl = slice(jj * P, (jj + 1) * P)
                    xsq = xsqpool.tile([P, KT, P], f32, tag="xsq")
                    _xsq_eng = nc.gpsimd if _XSQ_ENG == "gpsimd" else nc.vector
                    _xsq_eng.tensor_tensor(
                        xsq[:], xf[:, :, sl], xf[:, :, sl], op=MULT
                    )
                    t4 = xsqpool.tile([P, 4, P], f32, tag="t4")
                    nc.vector.tensor_tensor(
                        t4[:], xsq[:, 0:4, :], xsq[:, 4:8, :], op=mybir.AluOpType.add
                    )
                    t2 = xsqpool.tile([P, 2, P], f32, tag="t2")
                    nc.vector.tensor_tensor(
                        t2[:], t4[:, 0:2, :], t4[:, 2:4, :], op=mybir.AluOpType.add
                    )
                    t1 = xsqpool.tile([P, P], f32, tag="t1")
                    nc.vector.tensor_tensor(
                        t1[:], t2[:, 0, :], t2[:, 1, :], op=mybir.AluOpType.add
                    )
                    nc.tensor.matmul(
                        x2ps[:, jj:jj + 1],
                        t1[:],
                        negones_f[:],
                        start=True,
                        stop=True,
                    )
                # psum holds -sum x^2 per column; scale by 1/D on the way out
                nc.vector.tensor_scalar_mul(x2cols[:], x2ps[:], 1.0 / D)

                for jj in range(BCH // P):
                    j = ch * (BCH // P) + jj
                    y_t = ypool.tile([P, C], f32, tag="y_t")
                    ps0 = pmain.tile([P, 1024], f32, tag="ps")
                    ps1 = pmain.tile([P, 1024], f32, tag="ps")
                    pss = (ps0, ps0, ps1, ps1)
                    for k in range(KT):
                        lhsT = xbf[:, k, jj * P:(jj + 1) * P]
                        for cj in range(4):
                            nc.tensor.matmul(
                                pss[cj][:, (cj % 2) * 512:(cj % 2) * 512 + 512],
                                lhsT,
                                wbf[:, k, cj * 512:(cj + 1) * 512],
                                start=(k == 0),
                                stop=(k == KT - 1),
                            )
                    for h, psh in enumerate((ps0, ps1)):
                        t = epool.tile([P, 1024], f32, tag="t")
                        # t = (2/D)*psum - x2  (scale + per-partition bias)
                        nc.scalar.activation(
                            t[:],
                            psh[:],
                            Identity,
                            bias=x2cols[:, jj:jj + 1],
                            scale=2.0 / D,
                        )
                        # y = t - w2  (w2rep already negated)
                        nc.vector.tensor_add(
                            y_t[:, h * 1024:(h + 1) * 1024],
                            t[:],
                            w2rep[:, h * 1024:(h + 1) * 1024],
                        )
                    nc.sync.dma_start(y[j * P:(j + 1) * P, :], y_t[:])

    nc.compile()
    return nc


def _get_nc():
    if "nc" not in _CACHE:
        _CACHE["nc"] = _build_nc()
    return _CACHE["nc"]


def _prep_inputs(x, W):
    x = np.ascontiguousarray(x, dtype=np.float32)
    W = np.ascontiguousarray(W, dtype=np.float32)
    wT = np.ascontiguousarray(W.T).astype(ml_dtypes.bfloat16)
    in_maps = []
    for i in range(NCORES):
        xT_i = np.ascontiguousarray(x[i * BSH:(i + 1) * BSH, :].T)
        in_maps.append({"xT": xT_i, "wT": wT})
    return in_maps


def run(x, W, trace=False, **trace_kwargs):
    """Run on the 8 cores; returns (out [B, C] f32, BassKernelResults)."""
    from concourse import bass_utils

    nc = _get_nc()
    in_maps = _prep_inputs(x, W)
    res = bass_utils.run_bass_kernel_spmd(
        nc, in_maps, core_ids=list(range(NCORES)), trace=trace, **trace_kwargs
    )
    out = np.concatenate([r["y"] for r in res.results], axis=0)
    return out, res


def kernel(x, W, task_id=None, **_unused):
    out, _ = run(np.asarray(x), np.asarray(W), trace=False)
    return out


# revision 17
# speedup vs baseline: 1.0876x; 1.0876x over previous
"""EuclideanDeconf kernel for 8x TRN2 NeuronCores.

Computes out[b, c] = (2/D) * x @ W.T - ||x||^2/D - ||W||^2/D
for x [16384, 1024] f32, W [2048, 1024] f32 -> out [16384, 2048] f32.

Sharding: data-parallel over the batch dim. Each of the 8 cores gets 2048
rows of x (passed pre-transposed as xT [1024, 2048] f32) and the full W
(passed pre-transposed and bf16-cast as wT [1024, 2048] bf16). The host does
layout-only work (transpose / cast / shard / concat); all FLOPs (matmul,
row/col norms, combine) run on device.

Numerics: cross term in bf16 (its magnitude is ~0.003 of the ~1.0 output, so
bf16 rounding contributes ~1e-5 relative error); x2 computed on-device in
fp32 from the fp32 x (the dominant term, kept exact); w2 from bf16 W (w2 is
~0.002 so its rounding is ~1e-5 absolute).

Engine assignment (per core, steady state):
  PE:     512 bf16 matmuls [128x128]x[128x512] (the 8.6 GFLOP core of the op)
          + 32 w2-reduce + 4 w2-replicate + 16 tiny x2-dot matmuls
  ACT:    epilogue pass 1: t = (2/D)*psum - x2[b]  (scale + partition bias)
  DVE:    x casts, W^2 squares, x2 k-add-trees, epilogue pass 2: y = t - w2[c]
  GPSIMD: x^2 squares only
  DMA:    everything on the SP (sync) HWDGE ring; x chunk0 first, W second

All engines execute their queues in program order, so the w2 chain
(W DMA -> wsq -> reduce -> replicate) is laid out to never sit ahead of
chunk-0 work on any engine's queue, and x2 columns are produced per-b-tile
so ACT can drain PSUM as soon as each b-tile's accumulation closes.
"""

import numpy as np
import ml_dtypes

# Problem constants (hardcoded; kernel.py must be self-contained).
B, D, C = 16384, 1024, 2048
NCORES = 8
BSH = B // NCORES  # 2048 rows of x per core
P = 128            # partitions
KT = D // P        # 8 contraction tiles
BCH = 512          # b-chunk (columns of xT loaded per DMA)

_CACHE = {}


def _build_nc():
    import concourse.tile as tile
    import concourse.mybir as mybir
    import concourse.bass as bass
    from concourse import bacc

    f32 = mybir.dt.float32
    bf16 = mybir.dt.bfloat16
    PSUM = bass.MemorySpace.PSUM
    Identity = mybir.ActivationFunctionType.Identity
    MULT = mybir.AluOpType.mult
    ADD = mybir.AluOpType.add

    nc = bacc.Bacc(
        "TRN2",
        target_bir_lowering=False,
        debug=False,
        enable_asserts=False,
    )
    xT = nc.dram_tensor("xT", [D, BSH], f32, kind="ExternalInput").ap()
    wT = nc.dram_tensor("wT", [D, C], bf16, kind="ExternalInput").ap()
    y = nc.dram_tensor("y", [BSH, C], f32, kind="ExternalOutput").ap()

    with tile.TileContext(nc) as tc:
        with (
            tc.tile_pool(name="consts", bufs=1) as cpool,
            tc.tile_pool(name="wpool", bufs=1) as wpool,
            tc.tile_pool(name="xpool", bufs=2) as xpool,
            tc.tile_pool(name="xsqpool", bufs=2) as xsqpool,
            tc.tile_pool(name="epool", bufs=6) as epool,
            tc.tile_pool(name="ypool", bufs=2) as ypool,
            tc.tile_pool(name="spool", bufs=8) as spool,
            tc.tile_pool(name="pmain", bufs=3, space=PSUM) as pmain,
            tc.tile_pool(name="psmall", bufs=1, space=PSUM) as psmall,
        ):
            negones_f = cpool.tile([P, 1], f32)
            nc.gpsimd.memset(negones_f[:], -1.0)
            negones_b = cpool.tile([P, 1], bf16)
            nc.gpsimd.memset(negones_b[:], -1.0)
            ones1_b = cpool.tile([1, P], bf16)
            nc.gpsimd.memset(ones1_b[:], 1.0)
            warm = cpool.tile([1, 1], f32)
            # touch ACT early so its function-table DMA (~2.7us) is off the
            # critical path by the time the first epilogue runs
            nc.scalar.activation(warm[:], negones_f[0:1, 0:1], Identity,
                                 bias=0.0, scale=1.0)

            # ---- chunk 0 x load + cast, fine-grained so PE starts ASAP ----
            xTr0 = xT[:, 0:BCH].rearrange("(k p) b -> p k b", p=P)
            xf0 = xpool.tile([P, KT, BCH], f32, tag="xf")
            xbf0 = xpool.tile([P, KT, BCH], bf16, tag="xbf")
            for k in range(KT):
                nc.sync.dma_start(xf0[:, k, :], xTr0[:, k, :])
                nc.vector.tensor_copy(xbf0[:, k, :], xf0[:, k, :])

            # ---- W load (after chunk-0 x on the same ring) ----
            wbf = wpool.tile([P, KT, C], bf16)
            wTr = wT.rearrange("(k p) c -> p k c", p=P)
            for k in range(KT):
                nc.sync.dma_start(wbf[:, k, :], wTr[:, k, :])

            # ---- chunk 0 x^2 on gpsimd (its only job; starts right away) --
            xsq0 = []
            for jj in range(BCH // P):
                sl = slice(jj * P, (jj + 1) * P)
                xsq = xsqpool.tile([P, KT, P], f32, tag="xsq", name=f"xsq0_{jj}")
                nc.gpsimd.tensor_tensor(xsq[:], xf0[:, :, sl], xf0[:, :, sl],
                                        op=MULT)
                xsq0.append(xsq)

            y_bufs = {}

            def btile_matmuls(jg, xbf, jl):
                """Issue the 32 accumulating matmuls for one 128-row b-tile."""
                y_t = ypool.tile([P, C], f32, tag="y_t", name=f"y_t{jg}")
                ps0 = pmain.tile([P, 1024], f32, tag="ps", name=f"ps{jg}a")
                ps1 = pmain.tile([P, 1024], f32, tag="ps", name=f"ps{jg}b")
                pss = (ps0, ps0, ps1, ps1)
                for k in range(KT):
                    lhsT = xbf[:, k, jl * P:(jl + 1) * P]
                    for cj in range(4):
                        nc.tensor.matmul(
                            pss[cj][:, (cj % 2) * 512:(cj % 2) * 512 + 512],
                            lhsT,
                            wbf[:, k, cj * 512:(cj + 1) * 512],
                            start=(k == 0),
                            stop=(k == KT - 1),
                        )
                y_bufs[jg] = (y_t, ps0, ps1)

            def x2_col(xsq, tag):
                """x2 column (-sum(x^2)/D) for one b-tile: DVE tree + PE dot."""
                t4 = xsqpool.tile([P, 4, P], f32, tag="t4", name=f"t4_{tag}")
                nc.vector.tensor_tensor(t4[:], xsq[:, 0:4, :], xsq[:, 4:8, :],
                                        op=ADD)
                t2 = xsqpool.tile([P, 2, P], f32, tag="t2", name=f"t2_{tag}")
                nc.vector.tensor_tensor(t2[:], t4[:, 0:2, :], t4[:, 2:4, :],
                                        op=ADD)
                t1 = xsqpool.tile([P, P], f32, tag="t1", name=f"t1_{tag}")
                nc.vector.tensor_tensor(t1[:], t2[:, 0, :], t2[:, 1, :], op=ADD)
                x2ps = psmall.tile([P, 1], f32, tag="x2ps", bufs=1,
                                   name=f"x2ps{tag}")
                nc.tensor.matmul(x2ps[:], t1[:], negones_f[:],
                                 start=True, stop=True)
                x2c = spool.tile([P, 1], f32, tag="x2c", name=f"x2c{tag}")
                nc.vector.tensor_scalar_mul(x2c[:], x2ps[:], 1.0 / D)
                return x2c

            def btile_epilogue(jg, x2c, w2rep):
                y_t, ps0, ps1 = y_bufs.pop(jg)
                for h, psh in enumerate((ps0, ps1)):
                    t = epool.tile([P, 1024], f32, tag="t", name=f"t{jg}_{h}")
                    # t = (2/D)*psum - x2  (scale + per-partition bias)
                    nc.scalar.activation(t[:], psh[:], Identity,
                                         bias=x2c[:], scale=2.0 / D)
                    # y = t - w2  (w2rep already negated)
                    nc.vector.tensor_add(
                        y_t[:, h * 1024:(h + 1) * 1024],
                        t[:],
                        w2rep[:, h * 1024:(h + 1) * 1024],
                    )
                nc.sync.dma_start(y[jg * P:(jg + 1) * P, :], y_t[:])

            # ---- chunk 0: interleave b-tile matmuls with x2 columns ----
            x2c0 = []
            for jj in range(4):
                btile_matmuls(jj, xbf0, jj)
                x2c0.append(x2_col(xsq0[jj], f"c0_{jj}"))

            # ---- w2: squares (DVE), partition reduce + replicate (PE) ----
            w2rep = wpool.tile([P, C], f32)
            wsq = wpool.tile([P, KT, C], bf16)
            for k in range(KT):
                nc.vector.tensor_tensor(wsq[:, k, :], wbf[:, k, :],
                                        wbf[:, k, :], op=MULT)
            w2row = wpool.tile([1, C], bf16)
            for cj in range(C // 512):
                w2ps = psmall.tile([1, 512], f32, tag="w2ps", bufs=1,
                                   name=f"w2ps{cj}")
                for k in range(KT):
                    nc.tensor.matmul(
                        w2ps[:],
                        negones_b[:],
                        wsq[:, k, cj * 512:(cj + 1) * 512],
                        start=(k == 0),
                        stop=(k == KT - 1),
                    )
                # psum holds -sum W^2; scale by 1/D + cast to bf16 on copy-out
                nc.vector.tensor_scalar_mul(
                    w2row[:, cj * 512:(cj + 1) * 512], w2ps[:], 1.0 / D
                )
            for cj in range(C // 512):
                w2rp = psmall.tile([P, 512], f32, tag="w2ps", bufs=1,
                                   name=f"w2rp{cj}")
                nc.tensor.matmul(w2rp[:], ones1_b[:],
                                 w2row[:, cj * 512:(cj + 1) * 512],
                                 start=True, stop=True)
                nc.vector.tensor_copy(w2rep[:, cj * 512:(cj + 1) * 512],
                                      w2rp[:])

            # ---- chunk 0 epilogues ----
            for jj in range(4):
                btile_epilogue(jj, x2c0[jj], w2rep)

            # ---- chunks 1..3 ----
            for ch in range(1, BSH // BCH):
                xf = xpool.tile([P, KT, BCH], f32, tag="xf", name=f"xf{ch}")
                xbf = xpool.tile([P, KT, BCH], bf16, tag="xbf", name=f"xbf{ch}")
                xTr = xT[:, ch * BCH:(ch + 1) * BCH].rearrange(
                    "(k p) b -> p k b", p=P
                )
                nc.sync.dma_start(xf[:], xTr)
                nc.vector.tensor_copy(xbf[:], xf[:])

                for jj in range(4):
                    j = ch * 4 + jj
                    sl = slice(jj * P, (jj + 1) * P)
                    xsq = xsqpool.tile([P, KT, P], f32, tag="xsq",
                                       name=f"xsq{ch}_{jj}")
                    nc.gpsimd.tensor_tensor(xsq[:], xf[:, :, sl], xf[:, :, sl],
                                            op=MULT)
                    btile_matmuls(j, xbf, jj)
                    x2c = x2_col(xsq, f"c{ch}_{jj}")
                    btile_epilogue(j, x2c, w2rep)

    nc.compile()
    return nc


def _get_nc():
    if "nc" not in _CACHE:
        _CACHE["nc"] = _build_nc()
    return _CACHE["nc"]


def _prep_inputs(x, W):
    x = np.ascontiguousarray(x, dtype=np.float32)
    W = np.ascontiguousarray(W, dtype=np.float32)
    wT = np.ascontiguousarray(W.T).astype(ml_dtypes.bfloat16)
    in_maps = []
    for i in range(NCORES):
        xT_i = np.ascontiguousarray(x[i * BSH:(i + 1) * BSH, :].T)
        in_maps.append({"xT": xT_i, "wT": wT})
    return in_maps


def run(x, W, trace=False, **trace_kwargs):
    """Run on the 8 cores; returns (out [B, C] f32, BassKernelResults)."""
    from concourse import bass_utils

    nc = _get_nc()
    in_maps = _prep_inputs(x, W)
    res = bass_utils.run_bass_kernel_spmd(
        nc, in_maps, core_ids=list(range(NCORES)), trace=trace, **trace_kwargs
    )
    out = np.concatenate([r["y"] for r in res.results], axis=0)
    return out, res


def kernel(x, W, task_id=None, **_unused):
    out, _ = run(np.asarray(x), np.asarray(W), trace=False)
    return out


# revision 21
# speedup vs baseline: 1.1276x; 1.0367x over previous
"""EuclideanDeconf kernel for 8x TRN2 NeuronCores.

Computes out[b, c] = (2/D) * x @ W.T - ||x||^2/D - ||W||^2/D
for x [16384, 1024] f32, W [2048, 1024] f32 -> out [16384, 2048] f32.

Sharding: data-parallel over the batch dim. Each of the 8 cores gets 2048
rows of x (passed pre-transposed as xT [1024, 2048] f32) and the full W
(passed pre-transposed and bf16-cast as wT [1024, 2048] bf16). The host does
layout-only work (transpose / cast / shard / concat); all FLOPs (matmul,
row/col norms, combine) run on device.

Numerics: cross term in bf16 (its magnitude is ~0.003 of the ~1.0 output, so
bf16 rounding contributes ~1e-5 relative error); x2 computed on-device in
fp32 from the fp32 x (the dominant term, kept exact); w2 from bf16 W (w2 is
~0.002 so its rounding is ~1e-5 absolute).

Engine assignment (per core, steady state):
  PE:     512 bf16 matmuls [128x128]x[128x512] (the 8.6 GFLOP core of the op)
          + 32 w2-reduce + 4 w2-replicate + 16 tiny x2-dot matmuls
  ACT:    epilogue pass 1: t = (2/D)*psum - x2[b]  (scale + partition bias)
  DVE:    x casts, W^2 squares, x2 k-add-trees, epilogue pass 2: y = t - w2[c]
  GPSIMD: x^2 squares only
  DMA:    everything on the SP (sync) HWDGE ring; x chunk0 first, W second

All engines execute their queues in program order, so the w2 chain
(W DMA -> wsq -> reduce -> replicate) is laid out to never sit ahead of
chunk-0 work on any engine's queue, and x2 columns are produced per-b-tile
so ACT can drain PSUM as soon as each b-tile's accumulation closes.
"""

import numpy as np
import ml_dtypes

# Problem constants (hardcoded; kernel.py must be self-contained).
B, D, C = 16384, 1024, 2048
NCORES = 8
BSH = B // NCORES  # 2048 rows of x per core
P = 128            # partitions
KT = D // P        # 8 contraction tiles
BCH = 512          # b-chunk (columns of xT loaded per DMA)

_CACHE = {}


def _build_nc():
    import concourse.tile as tile
    import concourse.mybir as mybir
    import concourse.bass as bass
    from concourse import bacc

    f32 = mybir.dt.float32
    bf16 = mybir.dt.bfloat16
    PSUM = bass.MemorySpace.PSUM
    Identity = mybir.ActivationFunctionType.Identity
    Copy = mybir.ActivationFunctionType.Copy
    MULT = mybir.AluOpType.mult
    ADD = mybir.AluOpType.add

    nc = bacc.Bacc(
        "TRN2",
        target_bir_lowering=False,
        debug=False,
        enable_asserts=False,
    )
    xT = nc.dram_tensor("xT", [D, BSH], f32, kind="ExternalInput").ap()
    wT = nc.dram_tensor("wT", [D, C], bf16, kind="ExternalInput").ap()
    y = nc.dram_tensor("y", [BSH, C], f32, kind="ExternalOutput").ap()

    with tile.TileContext(nc) as tc:
        with (
            tc.tile_pool(name="consts", bufs=1) as cpool,
            tc.tile_pool(name="wpool", bufs=1) as wpool,
            tc.tile_pool(name="xpool", bufs=2) as xpool,
            tc.tile_pool(name="xsqpool", bufs=2) as xsqpool,
            tc.tile_pool(name="epool", bufs=6) as epool,
            tc.tile_pool(name="ypool", bufs=2) as ypool,
            tc.tile_pool(name="spool", bufs=8) as spool,
            tc.tile_pool(name="pmain", bufs=3, space=PSUM) as pmain,
            tc.tile_pool(name="psmall", bufs=1, space=PSUM) as psmall,
        ):
            negones_f = cpool.tile([P, 1], f32)
            nc.gpsimd.memset(negones_f[:], -1.0)
            negones_b = cpool.tile([P, 1], bf16)
            nc.gpsimd.memset(negones_b[:], -1.0)
            ones1_b = cpool.tile([1, P], bf16)
            nc.gpsimd.memset(ones1_b[:], 1.0)
            warm = cpool.tile([1, 1], f32)
            # touch ACT early so its function-table DMA (~2.7us) is off the
            # critical path by the time the first epilogue runs
            nc.scalar.activation(warm[:], negones_f[0:1, 0:1], Identity,
                                 bias=0.0, scale=1.0)

            # ---- PE warmup: dummy matmuls so HAM un-throttles (and the PE
            # is at 2.4 GHz) by the time real work arrives ----
            warm_b = cpool.tile([P, 512], bf16)
            nc.gpsimd.memset(warm_b[:], 0.0)
            warm_ps = psmall.tile([P, 512], f32, tag="w2ps", bufs=1)
            for _ in range(20):
                nc.tensor.matmul(warm_ps[:], warm_b[:, 0:P], warm_b[:],
                                 start=True, stop=True)

            # ---- chunk 0 x + W loads, interleaved per k-tile so the first
            # b-tile's matmuls (needing xbf[k] AND wbf[k]) start earliest ----
            xTr0 = xT[:, 0:BCH].rearrange("(k p) b -> p k b", p=P)
            xf0 = xpool.tile([P, KT, BCH], f32, tag="xf")
            xbf0 = xpool.tile([P, KT, BCH], bf16, tag="xbf")
            wbf = wpool.tile([P, KT, C], bf16)
            wTr = wT.rearrange("(k p) c -> p k c", p=P)
            for k in range(KT):
                nc.sync.dma_start(xf0[:, k, :], xTr0[:, k, :])
                nc.sync.dma_start(wbf[:, k, :], wTr[:, k, :])
                nc.vector.tensor_copy(xbf0[:, k, :], xf0[:, k, :])

            # ---- chunk 0 x^2 on gpsimd (its only job; starts right away) --
            xsq0 = []
            for jj in range(BCH // P):
                sl = slice(jj * P, (jj + 1) * P)
                xsq = xsqpool.tile([P, KT, P], f32, tag="xsq", name=f"xsq0_{jj}")
                nc.gpsimd.tensor_tensor(xsq[:], xf0[:, :, sl], xf0[:, :, sl],
                                        op=MULT)
                xsq0.append(xsq)

            y_bufs = {}

            def btile_matmuls(jg, xbf, jl):
                """Issue the 32 accumulating matmuls for one 128-row b-tile."""
                y_t = ypool.tile([P, C], f32, tag="y_t", name=f"y_t{jg}")
                ps0 = pmain.tile([P, 1024], f32, tag="ps", name=f"ps{jg}a")
                ps1 = pmain.tile([P, 1024], f32, tag="ps", name=f"ps{jg}b")
                pss = (ps0, ps0, ps1, ps1)
                for k in range(KT):
                    lhsT = xbf[:, k, jl * P:(jl + 1) * P]
                    for cj in range(4):
                        nc.tensor.matmul(
                            pss[cj][:, (cj % 2) * 512:(cj % 2) * 512 + 512],
                            lhsT,
                            wbf[:, k, cj * 512:(cj + 1) * 512],
                            start=(k == 0),
                            stop=(k == KT - 1),
                        )
                y_bufs[jg] = (y_t, ps0, ps1)

            def x2_col(xsq, tag):
                """x2 column (-sum(x^2)/D) for one b-tile: DVE tree + PE dot."""
                t4 = xsqpool.tile([P, 4, P], f32, tag="t4", name=f"t4_{tag}")
                nc.vector.tensor_tensor(t4[:], xsq[:, 0:4, :], xsq[:, 4:8, :],
                                        op=ADD)
                t2 = xsqpool.tile([P, 2, P], f32, tag="t2", name=f"t2_{tag}")
                nc.vector.tensor_tensor(t2[:], t4[:, 0:2, :], t4[:, 2:4, :],
                                        op=ADD)
                t1 = xsqpool.tile([P, P], f32, tag="t1", name=f"t1_{tag}")
                nc.vector.tensor_tensor(t1[:], t2[:, 0, :], t2[:, 1, :], op=ADD)
                x2ps = psmall.tile([P, 1], f32, tag="x2ps", bufs=1,
                                   name=f"x2ps{tag}")
                nc.tensor.matmul(x2ps[:], t1[:], negones_f[:],
                                 start=True, stop=True)
                x2c = spool.tile([P, 1], f32, tag="x2c", name=f"x2c{tag}")
                # copy-out on ACT (idle early; DVE is busy with casts/wsq)
                nc.scalar.activation(x2c[:], x2ps[:], Copy, bias=0.0,
                                     scale=1.0 / D)
                return x2c

            def btile_epilogue(jg, x2c, w2rep):
                y_t, ps0, ps1 = y_bufs.pop(jg)
                for h, psh in enumerate((ps0, ps1)):
                    t = epool.tile([P, 1024], f32, tag="t", name=f"t{jg}_{h}")
                    # t = (2/D)*psum - x2  (scale + per-partition bias)
                    nc.scalar.activation(t[:], psh[:], Identity,
                                         bias=x2c[:], scale=2.0 / D)
                    # y = t - w2  (w2rep already negated)
                    nc.vector.tensor_add(
                        y_t[:, h * 1024:(h + 1) * 1024],
                        t[:],
                        w2rep[:, h * 1024:(h + 1) * 1024],
                    )
                nc.sync.dma_start(y[jg * P:(jg + 1) * P, :], y_t[:])

            # ---- chunk 0: interleave b-tile matmuls with x2 columns ----
            x2c0 = []
            for jj in range(4):
                btile_matmuls(jj, xbf0, jj)
                x2c0.append(x2_col(xsq0[jj], f"c0_{jj}"))

            # ---- w2: squares (DVE), partition reduce + replicate (PE) ----
            w2rep = wpool.tile([P, C], f32)
            wsq = wpool.tile([P, KT, C], bf16)
            for k in range(KT):
                nc.vector.tensor_tensor(wsq[:, k, :], wbf[:, k, :],
                                        wbf[:, k, :], op=MULT)
            w2row = wpool.tile([1, C], bf16)
            for cj in range(C // 512):
                w2ps = psmall.tile([1, 512], f32, tag="w2ps", bufs=1,
                                   name=f"w2ps{cj}")
                for k in range(KT):
                    nc.tensor.matmul(
                        w2ps[:],
                        negones_b[:],
                        wsq[:, k, cj * 512:(cj + 1) * 512],
                        start=(k == 0),
                        stop=(k == KT - 1),
                    )
                # psum holds -sum W^2; scale by 1/D + cast to bf16 on copy-out
                # (ACT, which is idle early — DVE is busy with casts/wsq)
                nc.scalar.activation(w2row[:, cj * 512:(cj + 1) * 512],
                                     w2ps[:], Copy, bias=0.0, scale=1.0 / D)
            for cj in range(C // 512):
                w2rp = psmall.tile([P, 512], f32, tag="w2ps", bufs=1,
                                   name=f"w2rp{cj}")
                nc.tensor.matmul(w2rp[:], ones1_b[:],
                                 w2row[:, cj * 512:(cj + 1) * 512],
                                 start=True, stop=True)
                nc.scalar.activation(w2rep[:, cj * 512:(cj + 1) * 512],
                                     w2rp[:], Copy, bias=0.0, scale=1.0)

            # ---- chunk 0 epilogues ----
            for jj in range(4):
                btile_epilogue(jj, x2c0[jj], w2rep)

            # ---- chunks 1..3 ----
            for ch in range(1, BSH // BCH):
                xf = xpool.tile([P, KT, BCH], f32, tag="xf", name=f"xf{ch}")
                xbf = xpool.tile([P, KT, BCH], bf16, tag="xbf", name=f"xbf{ch}")
                xTr = xT[:, ch * BCH:(ch + 1) * BCH].rearrange(
                    "(k p) b -> p k b", p=P
                )
                nc.sync.dma_start(xf[:], xTr)
                nc.vector.tensor_copy(xbf[:], xf[:])

                for jj in range(4):
                    j = ch * 4 + jj
                    sl = slice(jj * P, (jj + 1) * P)
                    xsq = xsqpool.tile([P, KT, P], f32, tag="xsq",
                                       name=f"xsq{ch}_{jj}")
                    nc.gpsimd.tensor_tensor(xsq[:], xf[:, :, sl], xf[:, :, sl],
                                            op=MULT)
                    btile_matmuls(j, xbf, jj)
                    x2c = x2_col(xsq, f"c{ch}_{jj}")
                    btile_epilogue(j, x2c, w2rep)

    nc.compile()
    return nc


def _get_nc():
    if "nc" not in _CACHE:
        _CACHE["nc"] = _build_nc()
    return _CACHE["nc"]


def _prep_inputs(x, W):
    x = np.ascontiguousarray(x, dtype=np.float32)
    W = np.ascontiguousarray(W, dtype=np.float32)
    wT = np.ascontiguousarray(W.T).astype(ml_dtypes.bfloat16)
    in_maps = []
    for i in range(NCORES):
        xT_i = np.ascontiguousarray(x[i * BSH:(i + 1) * BSH, :].T)
        in_maps.append({"xT": xT_i, "wT": wT})
    return in_maps


def run(x, W, trace=False, **trace_kwargs):
    """Run on the 8 cores; returns (out [B, C] f32, BassKernelResults)."""
    from concourse import bass_utils

    nc = _get_nc()
    in_maps = _prep_inputs(x, W)
    res = bass_utils.run_bass_kernel_spmd(
        nc, in_maps, core_ids=list(range(NCORES)), trace=trace, **trace_kwargs
    )
    out = np.concatenate([r["y"] for r in res.results], axis=0)
    return out, res


def kernel(x, W, task_id=None, **_unused):
    out, _ = run(np.asarray(x), np.asarray(W), trace=False)
    return out
